# revision 1
# baseline (speedup 1.0000x reference)
"""Gram-stats variant: BN statistics computed from x on the PE, so the
BN scale/shift are ready before the main matmul outputs land and y PSUM
slots free right after the gating reads.

Per 512-row chunk (2 virtual batches):
  G_v   = sum_r x_r x_r^T     (PE, bf16 x from a casting DMA)    [128,128]
  mu_v  = sum_r x_r / 256     (PE ones-matmul on bf16 x)
  E[y^2]_c = w_c^T G w_c  via H'_T = W2pair_T^T G (PE bf16) and a
  diagonal extraction TTR against the pre-transposed W2T (DVE).
  var = E[y^2]/256 - mu_y^2;  rsqrt via float-domain quake seed + 2
  Newton iterations (DVE only).
  y_T = W2pair_T^T x (fp32r), sigmoid/relu with fused BN affine (ACT),
  products (DVE bf16), path-sum+fold+transpose via accumulating
  fold-matmuls (PE), contiguous store.
"""

import os
import sys

import numpy as np

if "/opt/trn_rl_repo" not in sys.path:
    sys.path.insert(0, "/opt/trn_rl_repo")

N_CORES = 8
B_FULL = 65536
B_CORE = B_FULL // N_CORES          # 8192
D_IN = 128
N_PATH = 8
C_TOT = 1024
VBS = 256
CHUNK = 512
N_CHUNK = B_CORE // CHUNK           # 16
BN_EPS = 1e-5

def _entmax15_np(x):
    """Exact entmax alpha=1.5 along last axis (numpy port of reference)."""
    x = np.asarray(x, np.float32)
    x = x - x.max(-1, keepdims=True)
    x = x / 2.0
    Xsrt = np.sort(x, -1)[..., ::-1].astype(np.float32)
    d = x.shape[-1]
    rho = np.arange(1, d + 1, dtype=np.float32)
    mean = np.cumsum(Xsrt, -1) / rho
    mean_sq = np.cumsum(Xsrt * Xsrt, -1) / rho
    ss = rho * (mean_sq - mean * mean)
    delta = np.clip((1.0 - ss) / rho, 0.0, None)
    tau = mean - np.sqrt(delta)
    support = (tau <= Xsrt).sum(-1, keepdims=True)
    tau_star = np.take_along_axis(tau, support - 1, axis=-1)
    return np.clip(x - tau_star, 0.0, None) ** 2



def _arrange_params(w2, gamma, beta):
    """Rearrange W2/gamma/beta into the pair-tile layout.

    Tile T (0..7): k = T//2 (path pair), ab = T%2 (0 = sigmoid half, 1 =
    linear half). Partition j of tile T holds channel
    c(T, j) = (2k + j//64)*128 + ab*64 + (j%64).
    w2_arr columns [T*128 + h*64 + o] = W2[:, (2k+h)*128 + ab*64 + o].
    gam/bet arranged [128, 16] with column T*2 + v (vb-duplicated).
    """
    w2_arr = np.empty_like(w2)
    g16 = np.empty((128, 16), np.float32)
    b16 = np.empty((128, 16), np.float32)
    for T in range(8):
        k, ab = T // 2, T % 2
        for h in range(2):
            path = 2 * k + h
            cols = slice(path * 128 + ab * 64, path * 128 + ab * 64 + 64)
            w2_arr[:, T * 128 + h * 64: T * 128 + h * 64 + 64] = w2[:, cols]
            for v in range(2):
                g16[h * 64:(h + 1) * 64, T * 2 + v] = gamma[cols]
                b16[h * 64:(h + 1) * 64, T * 2 + v] = beta[cols]
    return w2_arr, g16, b16



_BUILT = None


def _build_bass():
    import concourse.bacc as bacc
    import concourse.mybir as mybir
    from concourse.tile import TileContext
    from contextlib import ExitStack

    f32 = mybir.dt.float32
    f32r = mybir.dt.float32r
    bf16 = mybir.dt.bfloat16
    i32 = mybir.dt.int32
    AF = mybir.ActivationFunctionType
    OP = mybir.AluOpType

    nc = bacc.Bacc()

    x_d = nc.declare_dram_parameter("x", [B_CORE, D_IN], f32, isOutput=False)
    w2_d = nc.declare_dram_parameter("w2", [D_IN, C_TOT], f32r, isOutput=False)
    w2t_d = nc.declare_dram_parameter("w2t", [D_IN, C_TOT], f32, isOutput=False)
    gam_d = nc.declare_dram_parameter("gam16", [128, 16], f32, isOutput=False)
    bet_d = nc.declare_dram_parameter("bet16", [128, 16], f32, isOutput=False)
    aux_d = nc.declare_dram_parameter("aux", [128, 192], f32, isOutput=False)
    out_d = nc.declare_dram_parameter("out", [B_CORE, 64], f32, isOutput=True)

    with TileContext(nc) as tc, ExitStack() as es:
        cpool = es.enter_context(tc.tile_pool(name="consts", bufs=1))
        w2_sb = cpool.tile([128, C_TOT], f32r, tag="w2")
        w2b_sb = cpool.tile([128, C_TOT], bf16, tag="w2b")
        w2t_sb = cpool.tile([128, C_TOT], f32, tag="w2t")
        w2tb_sb = cpool.tile([128, C_TOT], bf16, tag="w2tb")
        gam_sb = cpool.tile([128, 16], f32, tag="gam")
        bet_sb = cpool.tile([128, 16], f32, tag="bet")
        aux_sb = cpool.tile([128, 192], f32, tag="aux")   # [I | fold]
        fb_sb = cpool.tile([128, 64], bf16, tag="fb")
        one_sb = cpool.tile([128, 1], f32, tag="oneb")

        nc.sync.dma_start(out=w2_sb[:], in_=w2_d[:, :])
        nc.sync.dma_start(out=w2t_sb[:], in_=w2t_d[:, :])
        nc.sync.dma_start(out=gam_sb[:], in_=gam_d[:, :])
        nc.sync.dma_start(out=bet_sb[:], in_=bet_d[:, :])
        nc.sync.dma_start(out=aux_sb[:], in_=aux_d[:, :])
        nc.vector.tensor_copy(fb_sb[:], aux_sb[:, 128:192])
        nc.vector.tensor_copy(w2b_sb[:], w2_sb[:].bitcast(f32))
        nc.vector.tensor_copy(w2tb_sb[:], w2t_sb[:])
        nc.vector.memset(one_sb[:], 1.0)

        ident = aux_sb[:, 0:128]
        w2r = w2_sb[:]

        xin_p = es.enter_context(tc.tile_pool(name="xin", bufs=4))
        xib_p = es.enter_context(tc.tile_pool(name="xib", bufs=4))
        xts_p = es.enter_context(tc.tile_pool(name="xts", bufs=4))
        g_p = es.enter_context(tc.tile_pool(name="gst", bufs=10))
        r_p = es.enter_context(tc.tile_pool(name="rst", bufs=10))
        pr_p = es.enter_context(tc.tile_pool(name="prod", bufs=10))
        st_p = es.enter_context(tc.tile_pool(name="stats", bufs=4))
        sc_p = es.enter_context(tc.tile_pool(name="scrap", bufs=4))
        gs_p = es.enter_context(tc.tile_pool(name="gsb", bufs=3))
        ot_p = es.enter_context(tc.tile_pool(name="otsb", bufs=4))

        yps_p = es.enter_context(tc.tile_pool(name="yps", bufs=4, space="PSUM"))
        hp_p = es.enter_context(tc.tile_pool(name="hp", bufs=2, space="PSUM"))
        msc_p = es.enter_context(tc.tile_pool(name="mscp", bufs=2, space="PSUM"))

        x_r = x_d[:, :].rearrange("(c t p) d -> c p t d", p=128, t=4)
        out_r = out_d[:, :].rearrange("(c t p) o -> c p t o", p=128, t=4)

        # PE warmups: absorb const-DMA sems into the PE clock one at a time
        # (each matmul instruction can carry only one sync wait).
        warm1 = msc_p.tile([128, 128], f32, tag="msc", name="warm1")
        nc.tensor.transpose(warm1[:], ident, ident)
        warm2 = msc_p.tile([128, 128], f32, tag="msc", name="warm2")
        nc.tensor.matmul(warm2[:], w2r[:, 0:128], w2r[:, 0:128],
                         start=True, stop=True)

        def emit_stats(c):
            """Chain producing scl/sh for chunk c (independent of y PSUM)."""
            xin = xin_p.tile([128, 4, 128], f32, tag="xin", name=f"xin{c}")
            nc.sync.dma_start(out=xin[:], in_=x_r[c])

            mgp = msc_p.tile([128, 260], f32, tag="msc", name=f"mgp{c}")
            for v in range(2):
                for t in range(2):
                    nc.tensor.matmul(mgp[:, 256 + v:257 + v],
                                     xin[:, 2 * v + t, :], one_sb[:],
                                     start=(t == 0), stop=(t == 1))
                for t in range(2):
                    nc.tensor.matmul(mgp[:, v * 128:(v + 1) * 128],
                                     xin[:, 2 * v + t, :], xin[:, 2 * v + t, :],
                                     start=(t == 0), stop=(t == 1))
            gsb = gs_p.tile([128, 256], bf16, tag="gsb", name=f"gsb{c}")
            nc.vector.tensor_copy(gsb[:], mgp[:, 0:256])
            mus = st_p.tile([128, 2], bf16, tag="mus", name=f"mus{c}")
            nc.vector.tensor_scalar_mul(mus[:], mgp[:, 256:258], 1.0 / VBS)

            muyp = msc_p.tile([128, 16], f32, tag="msc", name=f"muyp{c}")
            for T in range(8):
                nc.tensor.matmul(muyp[:, T * 2:T * 2 + 2],
                                 w2b_sb[:, T * 128:(T + 1) * 128], mus[:],
                                 start=True, stop=True)
            muy = st_p.tile([128, 16], f32, tag="muy", name=f"muy{c}")
            nc.vector.tensor_copy(muy[:], muyp[:])

            ss = st_p.tile([128, 16], f32, tag="ss", name=f"ss{c}")
            for half in range(4):
                hp = hp_p.tile([128, 4, 128], f32, tag="hp",
                               name=f"hp{c}_{half}")
                for q in range(4):
                    T, v = (half * 4 + q) // 2, (half * 4 + q) % 2
                    nc.tensor.matmul(hp[:, q, :],
                                     w2b_sb[:, T * 128:(T + 1) * 128],
                                     gsb[:, v * 128:(v + 1) * 128],
                                     start=True, stop=True)
                for q in range(4):
                    T, v = (half * 4 + q) // 2, (half * 4 + q) % 2
                    scr = sc_p.tile([128, 128], bf16, tag="scr",
                                    name=f"scr{c}_{half}_{q}")
                    nc.vector.scalar_tensor_tensor(
                        scr[:], hp[:, q, :], 1.0,
                        w2t_sb[:, T * 128:(T + 1) * 128],
                        OP.mult, OP.mult,
                        accum_out=ss[:, T * 2 + v:T * 2 + v + 1])

            vpe = st_p.tile([128, 16], f32, tag="vpe", name=f"vpe{c}")
            nc.vector.tensor_scalar(vpe[:], ss[:], 1.0 / VBS, BN_EPS,
                                    OP.mult, OP.add)
            msq = st_p.tile([128, 16], f32, tag="msq", name=f"msq{c}")
            nc.vector.tensor_mul(msq[:], muy[:], muy[:])
            nc.vector.tensor_sub(vpe[:], vpe[:], msq[:])
            rs = st_p.tile([128, 16], f32, tag="rs", name=f"rs{c}")
            nc.vector.tensor_scalar(rs[:].bitcast(i32), vpe[:].bitcast(i32),
                                    -0.5, 1597463007.0, OP.mult, OP.add)
            q_ = st_p.tile([128, 16], f32, tag="q", name=f"q{c}")
            for _ in range(2):
                nc.vector.tensor_mul(q_[:], rs[:], vpe[:])
                nc.vector.scalar_tensor_tensor(
                    q_[:], q_[:], -0.5, rs[:], OP.mult, OP.mult)
                nc.vector.scalar_tensor_tensor(
                    rs[:], q_[:], 1.5, rs[:], OP.add, OP.mult)
            scl = st_p.tile([128, 16], f32, tag="scl", name=f"scl{c}")
            nc.vector.tensor_mul(scl[:], rs[:], gam_sb[:])
            sh = st_p.tile([128, 16], f32, tag="sh", name=f"sh{c}")
            nc.vector.tensor_mul(sh[:], muy[:], scl[:])
            nc.vector.tensor_sub(sh[:], bet_sb[:], sh[:])
            return {"xin": xin, "scl": scl, "sh": sh}

        def emit_main(c, sd):
            xin, scl, sh = sd["xin"], sd["scl"], sd["sh"]
            xtp = yps_p.tile([128, 512], f32, tag="yps", name=f"xtp{c}")
            for t in range(4):
                nc.tensor.transpose(xtp[:, t * 128:(t + 1) * 128],
                                    xin[:, t, :], ident)
            xts = xts_p.tile([128, 512], f32r, tag="xts", name=f"xts{c}")
            nc.vector.tensor_copy(xts[:], xtp[:])
            xtr = xts[:]

            gt = [g_p.tile([128, 512], bf16, tag="gst", name=f"gst{c}_{i}")
                  for i in range(4)]
            rt = [r_p.tile([128, 512], bf16, tag="rst", name=f"rst{c}_{i}")
                  for i in range(4)]
            for T in range(8):
                yp = yps_p.tile([128, 512], f32, tag="yps", name=f"yps{c}_{T}")
                nc.tensor.matmul(yp[:], w2r[:, T * 128:(T + 1) * 128],
                                 xtr, start=True, stop=True)
                k = T // 2
                dst = gt[k] if T % 2 == 0 else rt[k]
                fn = AF.Sigmoid if T % 2 == 0 else AF.Relu
                for v in range(2):
                    col = T * 2 + v
                    nc.scalar.activation(
                        dst[:, v * 256:(v + 1) * 256],
                        yp[:, v * 256:(v + 1) * 256], fn,
                        bias=sh[:, col:col + 1], scale=scl[:, col:col + 1])

            prods = []
            for k in range(4):
                pr = pr_p.tile([128, 512], bf16, tag="prod", name=f"pr{c}_{k}")
                nc.gpsimd.tensor_mul(pr[:], gt[k][:], rt[k][:])
                prods.append(pr)
            return prods

        def emit_out(c, prods):
            otp = msc_p.tile([128, 256], f32, tag="msc", name=f"otp{c}")
            for t in range(4):
                for k in range(4):
                    nc.tensor.matmul(otp[:, t * 64:(t + 1) * 64],
                                     prods[k][:, t * 128:(t + 1) * 128],
                                     fb_sb[:], start=(k == 0), stop=(k == 3))
            ots = ot_p.tile([128, 4, 64], f32, tag="ots", name=f"ots{c}")
            nc.scalar.copy(ots[:], otp[:, 0:256])
            nc.sync.dma_start(out=out_r[c], in_=ots[:])

        # software pipeline: stats one chunk ahead of main; output stage
        # one chunk behind main (PE fold-matmuls no longer block the next
        # chunk's transposes in program order)
        pend = None
        pout = None
        for c in range(N_CHUNK + 2):
            if c < N_CHUNK:
                sd = emit_stats(c)
            npout = emit_main(c - 1, pend) if pend is not None else None
            if pout is not None:
                emit_out(c - 2, pout)
            pend = sd if c < N_CHUNK else None
            pout = npout

    nc.compile()
    return nc


def kernel(x, mask_w, conv_w, conv_b, gamma, beta):
    global _BUILT
    from concourse.bass_utils import run_bass_kernel_spmd

    x = np.asarray(x, np.float32)
    mask = _entmax15_np(np.asarray(mask_w, np.float32))
    w2 = (np.asarray(conv_w, np.float32) * mask[:, None, :]).transpose(2, 0, 1)
    w2 = np.ascontiguousarray(w2.reshape(D_IN, C_TOT), np.float32)
    w2a, g16, b16 = _arrange_params(w2, np.asarray(gamma, np.float32),
                                    np.asarray(beta, np.float32))
    # per-pair-tile transposed blocks for the diagonal extraction
    w2t = np.empty_like(w2a)
    for T in range(8):
        blk = w2a[:, T * 128:(T + 1) * 128]
        w2t[:, T * 128:(T + 1) * 128] = blk.T
    aux = np.zeros((128, 192), np.float32)
    aux[:, :128] = np.eye(128, dtype=np.float32)
    fold = np.zeros((128, 64), np.float32)
    fold[np.arange(128), np.arange(128) % 64] = 1.0
    aux[:, 128:] = fold

    if _BUILT is None:
        _BUILT = _build_bass()
    nc = _BUILT

    shards = x.reshape(N_CORES, B_CORE, D_IN)
    in_maps = [
        {"x": np.ascontiguousarray(shards[i]),
         "w2": np.ascontiguousarray(w2a), "w2t": np.ascontiguousarray(w2t),
         "gam16": np.ascontiguousarray(g16),
         "bet16": np.ascontiguousarray(b16), "aux": aux}
        for i in range(N_CORES)
    ]
    res = run_bass_kernel_spmd(nc, in_maps, list(range(N_CORES)))
    return np.concatenate([res.results[i]["out"] for i in range(N_CORES)], axis=0)

